# revision 41
# baseline (speedup 1.0000x reference)
"""Trainium2 Bass kernel for nn_MultiHeadGraphAttention.

Reference computation (B=4, N=2048, D=256, H=8, DK=32):
    Q = x @ w_q.T ; K = x @ w_k.T ; V = x @ w_v.T        (split into 8 heads of 32)
    scores = (Q K^T)/sqrt(32) + edge_weights, masked where mask==0
    out = softmax(scores) V  -> merge heads -> @ w_o.T + b_o
Sharding: 8 cores = batch(4) x sequence-halves(2). Each core owns batch b,
rows n0..n0+1023 and produces the full [1024, 256] output slab for them.

Per-core algorithm (transposed layout: scores_T[m=key, n=query]):
  The edge/mask term enters ADDITIVELY, pre-scaled on the host:
      emadd[m,n] = A16*edge[m,n]  (mask==1)  |  -1e30  (mask==0)
  and w_q carries A16/sqrt(DK), so the QK matmul PSUM holds A16*(s+e) after
  the emadd is added. Numerators are produced by one of three per-group
  flavors, chosen to balance the Vector/Scalar/Tensor engines:
    S  (DVE):  numer_i16 = int16(psum + B16 + emadd)  -- Schraudolph exp2
               bit-trick: the int16 bit pattern IS the bf16 numerator
               (~3% max elementwise err; masked entries saturate to -0.0).
    Ai (PE+ACT): identity-matmul injects emadd into PSUM before the QK
               accumulation, then ACT computes exp(psum/A16) exactly.
    Am (ACT+DVE): ACT computes exp(psum/A16) = exp(s), DVE multiplies by
               EM[m,n] = exp(emadd/A16) (one ACT op per key block).
  Attention @ V streams on the PE with a ones-column in V producing softmax
  denominators for free; normalization applies to the [256,1024] head output.
"""

import sys

for _p in ("/opt/trn_rl_repo", "/root/.axon_site/_ro/trn_rl_repo"):
    if _p not in sys.path:
        sys.path.insert(0, _p)

import numpy as np
import ml_dtypes

import concourse.bass as bass
import concourse.mybir as mybir
import concourse.tile as tile
from concourse.bass_utils import run_bass_kernel_spmd

B, N, D, H, DK = 4, 2048, 256, 8, 32
NL = N // 2          # rows per core
SCALE = float(np.sqrt(DK))
MB = N // 128        # 16 key blocks
NCH = NL // 512      # 2 query chunks of 512
F32 = mybir.dt.float32
BF16 = mybir.dt.bfloat16
I16 = mybir.dt.int16
F32NP = np.float32
BF16NP = ml_dtypes.bfloat16

A16 = 128.0 / float(np.log(2.0))
B16 = 16250.5
MASKED = -1e30

# numerator flavor per head: balances DVE (S) / ACT+PE (Ai) / ACT+DVE (Am)
FLAVORS = ("S", "Am", "S", "Am", "S", "Am", "S", "Am")

_wait_ctr = [0]


def _split_multi_waits(nc, max_waits=1):
    """Walrus in this container rejects >1 sync wait per instruction; move
    extra waits onto NOPs inserted just before, on the same engine."""
    for fn in nc.m.functions:
        for bb in fn.blocks:
            insts = bb.instructions
            out = []
            changed = False
            for inst in insts:
                si = inst.sync_info
                if si is not None and len(si.on_wait) > max_waits:
                    waits = list(si.on_wait)
                    for w in waits[:-max_waits]:
                        _wait_ctr[0] += 1
                        out.append(
                            mybir.InstNoOp(
                                name=f"waitsplit-nop-{_wait_ctr[0]}",
                                engine=inst.engine,
                                sync_info=mybir.SyncInfo(on_wait=[w], on_update=[]),
                            )
                        )
                    inst.sync_info = mybir.SyncInfo(
                        on_wait=waits[-max_waits:], on_update=list(si.on_update)
                    )
                    changed = True
                out.append(inst)
            if changed:
                insts.clear()
                insts.extend(out)


def _build_program():
    nc = bass.Bass()

    xT = nc.dram_tensor("xT", [D, N], BF16, kind="ExternalInput")
    xTq = nc.dram_tensor("xTq", [D, NL], BF16, kind="ExternalInput")
    emaddT = nc.dram_tensor("emaddT", [N, NL], BF16, kind="ExternalInput")
    wqT = nc.dram_tensor("wqT", [D, D], BF16, kind="ExternalInput")
    wkT = nc.dram_tensor("wkT", [D, D], BF16, kind="ExternalInput")
    wvT = nc.dram_tensor("wvT", [D, D], BF16, kind="ExternalInput")
    woT = nc.dram_tensor("woT", [D, D], BF16, kind="ExternalInput")
    ident = nc.dram_tensor("ident", [128, 128], BF16, kind="ExternalInput")
    bo = nc.dram_tensor("bo", [1, D], F32, kind="ExternalInput")
    outd = nc.dram_tensor("out", [NL, D], F32, kind="ExternalOutput")

    need_em = [False] * MB
    if "Am" in FLAVORS:
        for mb in range(MB):
            need_em[mb] = True

    with tile.TileContext(nc) as tc:
        with (
            tc.tile_pool(name="singles", bufs=1) as singles,
            tc.tile_pool(name="persist", bufs=1) as persist,
        ):
            # ---- static tiles -------------------------------------------------
            xT_sb = [singles.tile([128, N], BF16, name=f"xt{p}") for p in range(2)]
            xTq_sb = [singles.tile([128, NL], BF16, name=f"xtq{p}") for p in range(2)]
            wq_sb = [singles.tile([128, D], BF16, name=f"wq{p}") for p in range(2)]
            wk_sb = [singles.tile([128, D], BF16, name=f"wk{p}") for p in range(2)]
            wv_sb = [singles.tile([128, D], BF16, name=f"wv{p}") for p in range(2)]
            wo_sb = [singles.tile([128, D], BF16, name=f"wo{p}") for p in range(2)]
            bo_sb = singles.tile([128, D], F32, name="bo_sb")
            id_sb = singles.tile([128, 128], BF16, name="id_sb")
            ones128 = singles.tile([128, 32], F32, name="ones128")
            nc.vector.memset(ones128[:], 1.0)

            for p in range(2):
                nc.sync.dma_start(out=xT_sb[p][:], in_=xT[p * 128:(p + 1) * 128, :])
                nc.sync.dma_start(out=xTq_sb[p][:], in_=xTq[p * 128:(p + 1) * 128, :])
                nc.sync.dma_start(out=wq_sb[p][:], in_=wqT[p * 128:(p + 1) * 128, :])
                nc.sync.dma_start(out=wk_sb[p][:], in_=wkT[p * 128:(p + 1) * 128, :])
                nc.sync.dma_start(out=wv_sb[p][:], in_=wvT[p * 128:(p + 1) * 128, :])
                nc.sync.dma_start(out=wo_sb[p][:], in_=woT[p * 128:(p + 1) * 128, :])
            nc.sync.dma_start(out=id_sb[:], in_=ident[:, :])
            nc.gpsimd.dma_start(out=bo_sb[:], in_=bo[0:1, :].partition_broadcast(128))

            # persistent intermediates (Q/K in bf16: halves PE stream cost)
            QT_sb = [persist.tile([128, NL], BF16, name=f"qt{p}") for p in range(2)]
            KT_sb = [persist.tile([128, N], BF16, name=f"kt{p}") for p in range(2)]
            # V augmented with a ones column: AV matmul (M=33) then yields both
            # attention@V (rows 0-31) and the softmax denominator (row 32).
            V_aug = [persist.tile([128, H, 64], BF16, name=f"v_aug{mb}")
                     for mb in range(MB)]
            for mb in range(MB):
                nc.vector.memset(V_aug[mb][:], 0.0)
                nc.vector.memset(V_aug[mb][:, :, 32:33], 1.0)
            emadd_sb = [persist.tile([128, NL], BF16, name=f"ema{mb}")
                        for mb in range(MB)]
            em_sb = [persist.tile([128, NL], BF16, name=f"emt{mb}")
                     if need_em[mb] else None for mb in range(MB)]
            houtT = [[persist.tile([128, 512], BF16, name=f"ho{g}_{c}")
                      for c in range(NCH)] for g in range(2)]

            # ---- attention main loop -----------------------------------------
            with (
                tc.tile_pool(name="spool", bufs=4, space="PSUM") as spool,
                tc.tile_pool(name="avpool", bufs=1, space="PSUM") as avpool,
                tc.tile_pool(name="numpool", bufs=20) as numpool,
                tc.tile_pool(name="rcppool", bufs=2) as rcppool,
                tc.tile_pool(name="outpool", bufs=3) as outpool,
            ):
                def q_proj(p, f):
                    qps = spool.tile([128, 512], F32, name="qps", tag="s")
                    for dc in range(2):
                        nc.tensor.matmul(
                            qps[:],
                            wq_sb[dc][:, p * 128:(p + 1) * 128],
                            xTq_sb[dc][:, f * 512:(f + 1) * 512],
                            start=(dc == 0), stop=(dc == 1),
                        )
                    nc.scalar.copy(
                        QT_sb[p][:, f * 512:(f + 1) * 512], qps[:]
                    )

                def k_proj(p, f):
                    kps = spool.tile([128, 512], F32, name="kps", tag="s")
                    for dc in range(2):
                        nc.tensor.matmul(
                            kps[:],
                            wk_sb[dc][:, p * 128:(p + 1) * 128],
                            xT_sb[dc][:, f * 512:(f + 1) * 512],
                            start=(dc == 0), stop=(dc == 1),
                        )
                    nc.scalar.copy(
                        KT_sb[p][:, f * 512:(f + 1) * 512], kps[:]
                    )

                def v_proj(mb):
                    vps = spool.tile([128, D], F32, name="vps", tag="s")
                    for dc in range(2):
                        nc.tensor.matmul(
                            vps[:],
                            xT_sb[dc][:, mb * 128:(mb + 1) * 128],
                            wv_sb[dc][:],
                            start=(dc == 0), stop=(dc == 1),
                        )
                    nc.scalar.copy(
                        V_aug[mb][:, :, 0:32],
                        vps[:].rearrange("p (h d) -> p h d", h=H),
                    )

                def em_prep(mb):
                    nc.sync.dma_start(
                        out=emadd_sb[mb][:], in_=emaddT[mb * 128:(mb + 1) * 128, :]
                    )
                    if need_em[mb]:
                        # EM = exp(emadd/A16) -- masked entries underflow to 0
                        nc.scalar.activation(
                            em_sb[mb][:], emadd_sb[mb][:],
                            mybir.ActivationFunctionType.Exp,
                            bias=0.0, scale=1.0 / A16,
                        )

                for p in range(2):
                    for f in range(NCH):
                        q_proj(p, f)
                    k_proj(p, 0)
                for mb in range(4):
                    v_proj(mb)
                EM_PF = 3
                for mb in range(EM_PF):
                    em_prep(mb)
                pending = []
                for nch in range(NCH):
                    nsl = slice(nch * 512, (nch + 1) * 512)
                    if pending:
                        pending.pop(0)()
                    # bank b holds heads (2b, 2b+1): rows 0-32 and 64-96
                    avps = [
                        avpool.tile([128, 512], F32, name=f"av{b}", tag=f"av{b}")
                        for b in range(4)
                    ]
                    prev_avs = []
                    for mb in range(MB):
                        if nch == 0:
                            if mb % 4 == 2 and mb // 4 + 1 < 4:
                                k_proj(0, mb // 4 + 1)
                                k_proj(1, mb // 4 + 1)
                            if mb + 4 < MB:
                                v_proj(mb + 4)
                            if mb + EM_PF < MB:
                                em_prep(mb + EM_PF)
                        # per-head pipeline: 4 one-bank score slots; the AV
                        # matmuls for block mb issue during block mb+1, when
                        # their numerators are long done -- so the PE stream
                        # (scores, then zero-wait AVs) stays dense and the
                        # HAM clock-gate keeps the PE at full clock.
                        cur_avs = []
                        half_avs = [prev_avs[:4], prev_avs[4:]]
                        for h in range(H):
                            if h % 4 == 0:
                                # between 4-score bursts: AV block from the
                                # previous mb (numerators ready, no stall)
                                for amb, ah, numer in half_avs[h // 4]:
                                    b, sub = ah // 2, ah % 2
                                    nc.tensor.matmul(
                                        avps[b][64 * sub:64 * sub + 33, :],
                                        V_aug[amb][:, ah, 0:33],
                                        numer[:],
                                        start=(amb == 0),
                                        stop=(amb == MB - 1),
                                        tile_position=(0, 64 * sub),
                                    )
                            flav = FLAVORS[h]
                            sps = spool.tile([128, 512], F32, name="sps",
                                             tag="s")
                            if flav == "Ai":
                                nc.tensor.matmul(
                                    sps[:], id_sb[:], emadd_sb[mb][:, nsl],
                                    start=True, stop=False,
                                    skip_group_check=True,
                                )
                            # scores_T[m,n] = sum_dk KT[dk,m] QT[dk,n]
                            nc.tensor.matmul(
                                sps[:],
                                KT_sb[h // 4][(h % 4) * 32:(h % 4 + 1) * 32,
                                              mb * 128:(mb + 1) * 128],
                                QT_sb[h // 4][(h % 4) * 32:(h % 4 + 1) * 32,
                                              nsl],
                                start=(flav != "Ai"), stop=True,
                                tile_position=(32 * (h % 4), 0),
                                skip_group_check=(flav == "Ai"),
                            )
                            numer = numpool.tile([128, 512], BF16,
                                                 name="numer", tag="n")
                            if flav == "S":
                                # Schraudolph: int16(psum + B16 + emadd) bits
                                # ARE the bf16 numerator exp(s+e)
                                nc.vector.scalar_tensor_tensor(
                                    numer[:].bitcast(I16),
                                    sps[:],
                                    B16,
                                    emadd_sb[mb][:, nsl],
                                    op0=mybir.AluOpType.add,
                                    op1=mybir.AluOpType.add,
                                )
                            else:
                                nc.scalar.activation(
                                    numer[:], sps[:],
                                    mybir.ActivationFunctionType.Exp,
                                    bias=0.0, scale=1.0 / A16,
                                )
                                if flav == "Am":
                                    nc.vector.tensor_mul(
                                        numer[:], numer[:],
                                        em_sb[mb][:, nsl],
                                    )
                            cur_avs.append((mb, h, numer))
                        prev_avs = cur_avs
                    for amb, h, numer in prev_avs:
                        b, sub = h // 2, h % 2
                        nc.tensor.matmul(
                            avps[b][64 * sub:64 * sub + 33, :],
                            V_aug[amb][:, h, 0:33],
                            numer[:],
                            start=(amb == 0), stop=(amb == MB - 1),
                            tile_position=(0, 64 * sub),
                        )

                    # normalize: ACT reciprocal exp(-ln d) of denominator rows,
                    # PE ones-matmul broadcasts each row to its 32-row head
                    # block, then multiply the small [256,1024] head output.
                    def norm_and_proj(nch=nch, avps=avps):
                        rcpx = [
                            rcppool.tile([128, 512], F32, name=f"rcpx{b}",
                                         tag=f"rcpx{b % 2}")
                            for b in range(4)
                        ]
                        for b in range(4):
                            nc.scalar.activation(
                                rcpx[b][0:97, :], avps[b][0:97, :],
                                mybir.ActivationFunctionType.Ln,
                                bias=0.0, scale=1.0,
                            )
                            nc.scalar.activation(
                                rcpx[b][0:97, :], rcpx[b][0:97, :],
                                mybir.ActivationFunctionType.Exp,
                                bias=0.0, scale=-1.0,
                            )
                        for hg in range(2):
                            rcpb_ps = spool.tile(
                                [128, 512], F32, name="rcpb_ps", tag="s"
                            )
                            for j in range(4):
                                h = hg * 4 + j
                                b, sub = h // 2, h % 2
                                nc.tensor.matmul(
                                    rcpb_ps[32 * j:32 * j + 32, :],
                                    ones128[64 * sub + 32:64 * sub + 33, 0:32],
                                    rcpx[b][64 * sub + 32:64 * sub + 33, :],
                                    start=True, stop=True,
                                    tile_position=(64 * sub + 32, 32 * j),
                                )
                            rcpb_g = rcppool.tile(
                                [128, 512], F32, name=f"rcpb{hg}", tag=f"rcpb{hg}"
                            )
                            nc.scalar.copy(rcpb_g[:], rcpb_ps[:])
                            for j in range(4):
                                h = hg * 4 + j
                                b, sub = h // 2, h % 2
                                nc.vector.tensor_mul(
                                    houtT[hg][nch][32 * j:32 * j + 32, :],
                                    avps[b][64 * sub:64 * sub + 32, :],
                                    rcpb_g[32 * j:32 * j + 32, :],
                                )
                        for nbl in range(4):
                            nb = nch * 4 + nbl
                            ops = spool.tile([128, D], F32, name="ops", tag="s")
                            for g in range(2):
                                nc.tensor.matmul(
                                    ops[:],
                                    houtT[g][nch][:, nbl * 128:(nbl + 1) * 128],
                                    wo_sb[g][:],
                                    start=(g == 0), stop=(g == 1),
                                )
                            osb = outpool.tile([128, D], F32, name="osb", tag="osb")
                            nc.vector.tensor_add(osb[:], ops[:], bo_sb[:])
                            nc.sync.dma_start(
                                out=outd[nb * 128:(nb + 1) * 128, :], in_=osb[:]
                            )

                    pending.append(norm_and_proj)
                for fn in pending:
                    fn()

    _split_multi_waits(nc)
    return nc


_NC_CACHE = None


def _get_program():
    global _NC_CACHE
    if _NC_CACHE is None:
        _NC_CACHE = _build_program()
    return _NC_CACHE


def _make_in_maps(x, edge_weights, mask, w_q, w_k, w_v, w_o, b_o):
    wqT = np.ascontiguousarray((w_q * (A16 / SCALE)).T).astype(BF16NP)
    wkT = np.ascontiguousarray(w_k.T).astype(BF16NP)
    wvT = np.ascontiguousarray(w_v.T).astype(BF16NP)
    woT = np.ascontiguousarray(w_o.T).astype(BF16NP)
    bo = np.ascontiguousarray(b_o.reshape(1, D)).astype(F32NP)
    identity = np.eye(128, dtype=BF16NP)
    in_maps = []
    for c in range(8):
        b, half = c // 2, c % 2
        n0 = half * NL
        xTb = np.ascontiguousarray(x[b].T).astype(BF16NP)
        emadd = np.where(
            mask[b, n0:n0 + NL, :] == 1,
            (A16 * edge_weights[b, n0:n0 + NL, :]).astype(F32NP),
            F32NP(MASKED),
        ).T
        in_maps.append({
            "xT": xTb,
            "xTq": np.ascontiguousarray(xTb[:, n0:n0 + NL]),
            "emaddT": np.ascontiguousarray(emadd).astype(BF16NP),
            "wqT": wqT, "wkT": wkT, "wvT": wvT, "woT": woT,
            "ident": identity, "bo": bo,
        })
    return in_maps


def run_sharded(inputs, trace=False, tmpdir=None):
    """Run the SPMD kernel; returns (full_output, BassKernelResults)."""
    arrs = {k: np.asarray(v) for k, v in inputs.items()}
    nc = _get_program()
    in_maps = _make_in_maps(**arrs)
    res = run_bass_kernel_spmd(
        nc, in_maps, list(range(8)), trace=trace, tmpdir=tmpdir
    )
    out = np.empty((B, N, D), np.float32)
    for c in range(8):
        b, half = c // 2, c % 2
        out[b, half * NL:(half + 1) * NL, :] = res.results[c]["out"]
    return out, res


def kernel(**inputs):
    out, _ = run_sharded(inputs, trace=False)
    return out
